# revision 1
# baseline (speedup 1.0000x reference)
"""Causal self-attention (B=2, T=2048, C=1024, NH=16) on 8 trn2 NeuronCores.

Sharding: core c handles batch b = c//4 and head group g = c%4 (4 heads,
256 features). Each core computes q/k/v for its heads, causal attention in
S^T layout (keys on partitions, queries on the free dim), and a partial
output projection  y_heads @ w_proj[head_rows, :].  The host sums the four
partial projections per batch and adds b_proj.

Kernel layout notes:
  - X^T ([C, T], C on partitions) is produced on-chip with PE transposes.
  - Q^T/K^T are computed as [feat, T] tiles (2 heads per 128-partition tile),
    V as [T, feat] (natural), which feeds every later matmul without any
    further transposes:
      S^T[k, q]   = K^T-tile.T @ Q^T     (two heads packed in the PE rows)
      P^T         = exp(S^T * 1/8)       (no max subtraction; scores ~ N(0,1))
      y^T[d, q]   = V-tile.T @ P^T       (two heads packed in the PE cols)
      sums[1, q]  = ones.T @ P^T         (packed in spare PE col strips)
      out[t, o]   = y^T-tile.T @ w_proj  (partial; host reduces over cores)
  - All matmuls run as float32r (full-rate fp32 PE mode).
"""

import os
import sys

import numpy as np

for _p in ("/opt/trn_rl_repo", "/root/.axon_site/_ro/trn_rl_repo"):
    if _p not in sys.path and os.path.isdir(_p):
        sys.path.append(_p)

import concourse.bass as bass  # noqa: E402
import concourse.tile as tile  # noqa: E402
from concourse import bacc, mybir  # noqa: E402
from concourse.bass_utils import run_bass_kernel_spmd  # noqa: E402

P = 128
B, T, C = 2, 2048, 1024
NH, HD = 16, 64
HPC = 4  # heads per core
FPC = HPC * HD  # features per core (256)
QCW = 512  # query-chunk width (max fp32 moving dim)
F32 = mybir.dt.float32
F32R = mybir.dt.float32r
BF16 = mybir.dt.bfloat16


def build_nc(t_len: int = T, debug: bool = False):
    """Build the per-core Bass program (same program on all 8 cores)."""
    nt = t_len // P  # token tiles
    ncb = C // P  # contraction blocks
    nqc = t_len // QCW  # query chunks

    nc = bacc.Bacc("TRN2", target_bir_lowering=False, debug=False)

    x_d = nc.dram_tensor("x", [t_len, C], F32, kind="ExternalInput")
    wq_d = nc.dram_tensor("wq", [C, FPC], F32R, kind="ExternalInput")
    wk_d = nc.dram_tensor("wk", [C, FPC], F32R, kind="ExternalInput")
    wv_d = nc.dram_tensor("wv", [C, FPC], F32R, kind="ExternalInput")
    bqkv_d = nc.dram_tensor("bqkv", [3, FPC], F32R, kind="ExternalInput")
    wp_d = nc.dram_tensor("wp", [FPC, C], F32R, kind="ExternalInput")
    triu_d = nc.dram_tensor("triu", [P, P], F32R, kind="ExternalInput")
    ident_d = nc.dram_tensor("ident", [P, P], F32, kind="ExternalInput")
    ones_d = nc.dram_tensor("ones", [P, QCW], F32R, kind="ExternalInput")
    out_d = nc.dram_tensor("out", [t_len, C], F32, kind="ExternalOutput")

    from contextlib import ExitStack

    with tile.TileContext(nc) as tc, ExitStack() as ctx:
            consts = ctx.enter_context(tc.tile_pool(name="consts", bufs=1))
            stage = ctx.enter_context(tc.tile_pool(name="stage", bufs=2))
            bigs = ctx.enter_context(tc.tile_pool(name="bigs", bufs=1))
            xts = ctx.enter_context(tc.tile_pool(name="xts", bufs=ncb))
            qkts = ctx.enter_context(tc.tile_pool(name="qkts", bufs=4))
            yts = ctx.enter_context(tc.tile_pool(name="yts", bufs=2))
            exps = ctx.enter_context(tc.tile_pool(name="exps", bufs=3))
            smalls = ctx.enter_context(tc.tile_pool(name="smalls", bufs=1))
            psum = ctx.enter_context(tc.tile_pool(name="psum", bufs=8, space="PSUM"))
            # ---- constants / weights into SBUF ----
            triu = consts.tile([P, P], F32R)
            ident = consts.tile([P, P], F32)
            ones = consts.tile([P, QCW], F32R)
            nc.sync.dma_start(out=triu, in_=triu_d.ap())
            nc.sync.dma_start(out=ident, in_=ident_d.ap())
            nc.sync.dma_start(out=ones, in_=ones_d.ap())

            b_sb = []
            for i in range(3):
                bt = consts.tile([1, FPC], F32R, tag=f"bias{i}")
                nc.sync.dma_start(out=bt, in_=bqkv_d.ap()[i : i + 1, :])
                b_sb.append(bt)

            wq_sb = bigs.tile([P, ncb, FPC], F32R, tag="wq")
            wk_sb = bigs.tile([P, ncb, FPC], F32R, tag="wk")
            wv_sb = bigs.tile([P, ncb, FPC], F32R, tag="wv")
            for wsb, wd in ((wq_sb, wq_d), (wk_sb, wk_d), (wv_sb, wv_d)):
                nc.sync.dma_start(
                    out=wsb, in_=wd.ap().rearrange("(cb p) f -> p cb f", p=P)
                )
            wp_sb = bigs.tile([P, 2, C], F32R, tag="wp")
            nc.sync.dma_start(
                out=wp_sb, in_=wp_d.ap().rearrange("(fb p) o -> p fb o", p=P)
            )

            # ---- phase 1: X^T via PE transposes ----
            xt = [xts.tile([P, t_len], F32R, tag="xt", name=f"xt{i}") for i in range(ncb)]
            for t in range(nt):
                xst = stage.tile([P, C], F32, tag="stage")
                nc.sync.dma_start(out=xst, in_=x_d.ap()[t * P : (t + 1) * P, :])
                for cb in range(ncb):
                    ps = psum.tile([P, P], F32, tag="ps")
                    nc.tensor.transpose(ps, xst[:, cb * P : (cb + 1) * P], ident)
                    nc.vector.tensor_copy(
                        out=xt[cb][:, t * P : (t + 1) * P], in_=ps
                    )

            # ---- phase 2: Q^T, K^T ([feat, T], 2 heads/tile), V ([T, feat]) --
            qt = [qkts.tile([P, t_len], F32R, tag="qkt", name=f"qt{i}") for i in range(2)]
            kt = [qkts.tile([P, t_len], F32R, tag="qkt", name=f"kt{i}") for i in range(2)]
            for widx, wsb, dst, scale in (
                (0, wq_sb, qt, 0.125),
                (1, wk_sb, kt, None),
            ):
                for pair in range(2):
                    fs = slice(pair * P, (pair + 1) * P)
                    for qc in range(nqc):
                        cs = slice(qc * QCW, (qc + 1) * QCW)
                        ps = psum.tile([P, QCW], F32, tag="ps")
                        for cb in range(ncb):
                            nc.tensor.matmul(
                                ps,
                                (wsb[:, cb, fs]),
                                (xt[cb][:, cs]),
                                start=(cb == 0),
                                stop=False,
                            )
                        nc.tensor.matmul(
                            ps,
                            (b_sb[widx][0:1, fs]),
                            (ones[0:1, :]),
                            start=False,
                            stop=True,
                        )
                        if scale is not None:
                            nc.vector.tensor_scalar_mul(dst[pair][:, cs], ps, scale)
                        else:
                            nc.vector.tensor_copy(out=dst[pair][:, cs], in_=ps)

            # V stored as [P, nt, pair, 130]: per pair, head-A block cols 0:65
            # = [d(64), ones], head-B block cols 65:130 = [d(64), ones].  The
            # ones column makes the PV matmul also produce the softmax
            # denominator in output row 64 (M=65).
            v_sb = bigs.tile([P, nt, 2, 130], F32R, tag="v")
            for h in (64, 129):
                nc.vector.tensor_copy(
                    out=v_sb[:, :, :, h],
                    in_=ones[:, 0 : nt * 2].rearrange("p (a b) -> p a b", b=2),
                )
            for t in range(nt):
                ps = psum.tile([P, FPC], F32, tag="ps")
                for cb in range(ncb):
                    nc.tensor.matmul(
                        ps,
                        (xt[cb][:, t * P : (t + 1) * P]),
                        (wv_sb[:, cb, :]),
                        start=(cb == 0),
                        stop=False,
                    )
                nc.tensor.matmul(
                    ps,
                    (ones[0:1, 0:P]),
                    (b_sb[2][0:1, :]),
                    start=False,
                    stop=True,
                )
                nc.vector.tensor_copy(
                    out=v_sb[:, t].rearrange("p a (h w) -> p a h w", w=65)[
                        :, :, :, 0:64
                    ],
                    in_=ps.rearrange("p (a h w) -> p a h w", a=2, w=64),
                )

            # ---- phase 3: causal attention in S^T layout ----
            yt = [yts.tile([P, t_len], F32R, tag="yt", name=f"yt{i}") for i in range(2)]
            for pair in range(2):
                for qc in range(nqc):
                    cs = slice(qc * QCW, (qc + 1) * QCW)
                    cs0 = qc * QCW
                    nki = 4 * (qc + 1)
                    yA_ps = psum.tile([P, QCW], F32, tag="ps", name="yA_ps")
                    yB_ps = psum.tile([P, QCW], F32, tag="ps", name="yB_ps")
                    for ki in range(nki):
                        m = ki - 4 * qc
                        lo = max(m, 0) * P  # first unmasked column of this k-tile
                        ks = slice(ki * P, (ki + 1) * P)
                        stA = psum.tile([P, QCW], F32, tag="ps", name="stA")
                        stB = psum.tile([P, QCW], F32, tag="ps", name="stB")
                        nc.tensor.matmul(
                            stA[:, lo:],
                            kt[pair][0:64, ks],
                            qt[pair][0:64, cs0 + lo : cs0 + QCW],
                            start=True,
                            stop=True,
                        )
                        nc.tensor.matmul(
                            stB[:, lo:],
                            kt[pair][64:P, ks],
                            qt[pair][64:P, cs0 + lo : cs0 + QCW],
                            start=True,
                            stop=True,
                            tile_position=(64, 0),
                        )
                        eA = exps.tile([P, QCW], F32R, tag="exp", name="eA")
                        eB = exps.tile([P, QCW], F32R, tag="exp", name="eB")
                        nc.scalar.activation(
                            eA[:, lo:], stA[:, lo:], mybir.ActivationFunctionType.Exp
                        )
                        nc.scalar.activation(
                            eB[:, lo:], stB[:, lo:], mybir.ActivationFunctionType.Exp
                        )
                        if m >= 0:  # diagonal 128-block: causal triangle mask
                            ds_ = slice(m * P, (m + 1) * P)
                            nc.vector.tensor_mul(eA[:, ds_], eA[:, ds_], triu)
                            nc.vector.tensor_mul(eB[:, ds_], eB[:, ds_], triu)
                        if debug and pair == 0 and qc == 0 and ki in (0, 3):
                            dbgE = smalls.tile(
                                [P, QCW], F32R, tag=f"dbgE{ki}", bufs=1,
                                name=f"dbgE{ki}",
                            )
                            nc.vector.tensor_copy(out=dbgE[:, lo:], in_=eA[:, lo:])
                            d = nc.dram_tensor(
                                f"dbg_e{ki}", [P, QCW], F32R, kind="ExternalOutput"
                            )
                            nc.sync.dma_start(out=d.ap(), in_=dbgE)
                        st, sp = ki == 0, ki == nki - 1
                        nc.tensor.matmul(
                            yA_ps[0:65, lo:],
                            v_sb[:, ki, pair, 0:65],
                            eA[:, lo:],
                            start=st,
                            stop=sp,
                        )
                        nc.tensor.matmul(
                            yB_ps[0:65, lo:],
                            v_sb[:, ki, pair, 65:130],
                            eB[:, lo:],
                            start=st,
                            stop=sp,
                        )
                    if debug and pair == 0 and qc == 0:
                        for nm, src in (("dbg_ya", yA_ps), ("dbg_yb", yB_ps)):
                            dbgY = smalls.tile(
                                [P, QCW], F32, tag=nm, bufs=1, name=nm
                            )
                            nc.vector.tensor_copy(
                                out=dbgY[0:65, :], in_=src[0:65, :]
                            )
                            d = nc.dram_tensor(
                                nm, [P, QCW], F32, kind="ExternalOutput"
                            )
                            nc.sync.dma_start(out=d.ap(), in_=dbgY)
                    # Copy unnormalized y (+ sums in row 64) to SBUF right
                    # away so the PSUM banks free up for the next iteration's
                    # matmuls (the in-order PE queue stalls on slot waits).
                    yuA = smalls.tile([65, QCW], F32, tag="yuA")
                    yuB = smalls.tile([65, QCW], F32, tag="yuB")
                    nc.vector.tensor_copy(out=yuA, in_=yA_ps[0:65, :])
                    nc.vector.tensor_copy(out=yuB, in_=yB_ps[0:65, :])
                    nc.vector.reciprocal(yuA[64:65, :], yuA[64:65, :])
                    nc.vector.reciprocal(yuB[64:65, :], yuB[64:65, :])
                    recbA = smalls.tile([64, QCW], F32, tag="recbA")
                    recbB = smalls.tile([64, QCW], F32, tag="recbB")
                    nc.gpsimd.dma_start(
                        out=recbA,
                        in_=yuA[64:65, None, :].broadcast_to([1, 64, QCW]),
                    )
                    nc.gpsimd.dma_start(
                        out=recbB,
                        in_=yuB[64:65, None, :].broadcast_to([1, 64, QCW]),
                    )
                    if debug and pair == 0 and qc == 0:
                        dbgR = smalls.tile(
                            [P, QCW], F32, tag="dbg_recb", bufs=1, name="dbgR"
                        )
                        nc.vector.tensor_copy(out=dbgR[0:64, :], in_=recbA)
                        nc.vector.tensor_copy(out=dbgR[64:P, :], in_=recbB)
                        d = nc.dram_tensor(
                            "dbg_recb", [P, QCW], F32, kind="ExternalOutput"
                        )
                        nc.sync.dma_start(out=d.ap(), in_=dbgR)
                    nc.vector.tensor_mul(
                        yt[pair][0:64, cs], yuA[0:64, :], recbA
                    )
                    nc.vector.tensor_mul(
                        yt[pair][64:P, cs], yuB[0:64, :], recbB
                    )

            if debug:
                dbg_specs = [
                    ("dbg_xt", xt[0]),
                    ("dbg_qt", qt[0]),
                    ("dbg_kt", kt[0]),
                    ("dbg_yt", yt[0]),
                    ("dbg_v", v_sb.rearrange("p a b c -> p (a b c)")),
                ]
                for nm, src in dbg_specs:
                    d = nc.dram_tensor(
                        nm, [P, src.free_size()], src.dtype, kind="ExternalOutput"
                    )
                    nc.sync.dma_start(out=d.ap(), in_=src)

            # ---- phase 4: partial output projection ----
            for t in range(nt):
                ost = stage.tile([P, C], F32, tag="stage")
                for nch in range(2):
                    ps = psum.tile([P, QCW], F32, tag="ps")
                    for fb in range(2):
                        nc.tensor.matmul(
                            ps,
                            (yt[fb][:, t * P : (t + 1) * P]),
                            (wp_sb[:, fb, nch * QCW : (nch + 1) * QCW]),
                            start=(fb == 0),
                            stop=(fb == 1),
                        )
                    nc.vector.tensor_copy(
                        out=ost[:, nch * QCW : (nch + 1) * QCW], in_=ps
                    )
                nc.sync.dma_start(out=out_d.ap()[t * P : (t + 1) * P, :], in_=ost)

    nc.compile()
    return nc


_NC_CACHE: dict = {}
LAST_RESULT = None


def kernel(x, w_attn, b_attn, w_proj, b_proj):
    global LAST_RESULT
    x = np.ascontiguousarray(np.asarray(x, np.float32))
    w_attn = np.ascontiguousarray(np.asarray(w_attn, np.float32))
    b_attn = np.ascontiguousarray(np.asarray(b_attn, np.float32))
    w_proj = np.ascontiguousarray(np.asarray(w_proj, np.float32))
    b_proj = np.ascontiguousarray(np.asarray(b_proj, np.float32))

    if "nc" not in _NC_CACHE:
        _NC_CACHE["nc"] = build_nc(T)
    nc = _NC_CACHE["nc"]

    triu = np.triu(np.ones((P, P), np.float32))
    ident = np.eye(P, dtype=np.float32)
    ones = np.ones((P, QCW), np.float32)

    in_maps = []
    for core in range(8):
        b, g = core // 4, core % 4
        f0 = g * FPC
        in_maps.append(
            {
                "x": np.ascontiguousarray(x[b]),
                "wq": np.ascontiguousarray(w_attn[:, f0 : f0 + FPC]),
                "wk": np.ascontiguousarray(w_attn[:, C + f0 : C + f0 + FPC]),
                "wv": np.ascontiguousarray(
                    w_attn[:, 2 * C + f0 : 2 * C + f0 + FPC]
                ),
                "bqkv": np.stack(
                    [
                        b_attn[f0 : f0 + FPC],
                        b_attn[C + f0 : C + f0 + FPC],
                        b_attn[2 * C + f0 : 2 * C + f0 + FPC],
                    ]
                ),
                "wp": np.ascontiguousarray(w_proj[f0 : f0 + FPC, :]),
                "triu": triu,
                "ident": ident,
                "ones": ones,
            }
        )

    trace = bool(os.environ.get("BASS_TRACE"))
    res = run_bass_kernel_spmd(
        nc,
        in_maps,
        core_ids=list(range(8)),
        trace=trace,
        tmpdir=os.environ.get("KERNEL_TRACE_DIR") or None,
    )
    LAST_RESULT = res

    y = np.empty((B, T, C), np.float32)
    for b in range(B):
        acc = res.results[4 * b]["out"].astype(np.float32).copy()
        for g in range(1, 4):
            acc += res.results[4 * b + g]["out"]
        y[b] = acc + b_proj[None, :]
    return y



# revision 5
# speedup vs baseline: 1.4257x; 1.4257x over previous
"""Causal self-attention (B=2, T=2048, C=1024, NH=16) on 8 trn2 NeuronCores.

Sharding: core c handles batch b = c//4 and head group g = c%4 (4 heads,
256 features). Each core computes q/k/v for its heads, causal attention in
S^T layout (keys on partitions, queries on the free dim), and a partial
output projection  y_heads @ w_proj[head_rows, :].  The host sums the four
partial projections per batch and adds b_proj.

v2 design (vs the fp32r baseline):
  - All matmul operands are bf16 (PSUM accumulation stays fp32).  bf16 runs
    at 1 cycle/row for every moving width, enables FWL weight loads, and
    halves DMA/SBUF traffic.
  - x is transposed on the HOST (numpy) -> no PE transpose phase at all.
    All dram layouts are pre-arranged so every DMA is fully contiguous.
  - Loops are emitted qc-chunk-at-a-time (QKV(qc+1) is emitted before
    proj(qc), S(ki+1) before PV(ki)) so the in-order PE queue never sits
    behind an instruction that waits on scalar/vector work -> HAM clock
    gate stays released (2.4 GHz).
  - Softmax: exp on the scalar engine with scale=1/8 folded in; denominator
    comes free from a ones-column packed into V (PSUM row 64); reciprocal
    is a [1,512] DVE op; normalization multiplies straight out of PSUM.
  - QKV biases are folded into the PSUM->SBUF copies (tensor_scalar_add
    with a per-partition bias AP); V bias via a 1-row broadcast matmul.
"""

import os
import sys

import numpy as np
import ml_dtypes

for _p in ("/opt/trn_rl_repo", "/root/.axon_site/_ro/trn_rl_repo"):
    if _p not in sys.path and os.path.isdir(_p):
        sys.path.append(_p)

import concourse.bass as bass  # noqa: E402
import concourse.tile as tile  # noqa: E402
from concourse import bacc, mybir  # noqa: E402
from concourse.bass_utils import run_bass_kernel_spmd  # noqa: E402

P = 128
B, T, C = 2, 2048, 1024
NH, HD = 16, 64
HPC = 4  # heads per core
FPC = HPC * HD  # features per core (256)
QCW = 512  # query-chunk width (PSUM bank = 512 fp32)
NQC = T // QCW
NT = T // P
NCB = C // P
F32 = mybir.dt.float32
BF16 = mybir.dt.bfloat16
BF = ml_dtypes.bfloat16
EXP = mybir.ActivationFunctionType.Exp


def build_nc():
    nc = bacc.Bacc("TRN2", target_bir_lowering=False, debug=False)

    x_d = nc.dram_tensor("x", [C, T], BF16, kind="ExternalInput")  # x^T
    wq_d = nc.dram_tensor("wq", [P, NCB, FPC], BF16, kind="ExternalInput")
    wk_d = nc.dram_tensor("wk", [P, NCB, FPC], BF16, kind="ExternalInput")
    wv_d = nc.dram_tensor("wv", [P, NCB, FPC], BF16, kind="ExternalInput")
    wp_d = nc.dram_tensor("wp", [P, 2, C], BF16, kind="ExternalInput")
    bq_d = nc.dram_tensor("bq", [P, 2], F32, kind="ExternalInput")
    bk_d = nc.dram_tensor("bk", [P, 2], F32, kind="ExternalInput")
    bv_d = nc.dram_tensor("bv", [1, FPC], BF16, kind="ExternalInput")
    triu_d = nc.dram_tensor("triu", [P, P], BF16, kind="ExternalInput")
    ones1_d = nc.dram_tensor("ones1", [1, P], BF16, kind="ExternalInput")
    out_d = nc.dram_tensor("out", [T, C], F32, kind="ExternalOutput")

    from contextlib import ExitStack

    with tile.TileContext(nc) as tc, ExitStack() as ctx:
        consts = ctx.enter_context(tc.tile_pool(name="consts", bufs=1))
        bigs = ctx.enter_context(tc.tile_pool(name="bigs", bufs=1))
        xts = ctx.enter_context(tc.tile_pool(name="xts", bufs=NCB))
        exps = ctx.enter_context(tc.tile_pool(name="exps", bufs=6))
        smalls = ctx.enter_context(tc.tile_pool(name="smalls", bufs=2))
        recs = ctx.enter_context(tc.tile_pool(name="recs", bufs=2))
        recbs = ctx.enter_context(tc.tile_pool(name="recbs", bufs=2))
        stage = ctx.enter_context(tc.tile_pool(name="stage", bufs=3))
        psum = ctx.enter_context(tc.tile_pool(name="psum", bufs=8, space="PSUM"))

        # ---- weights / consts into SBUF (all DMAs fully contiguous) ----
        wq_sb = bigs.tile([P, NCB, FPC], BF16, tag="wq")
        wk_sb = bigs.tile([P, NCB, FPC], BF16, tag="wk")
        nc.sync.dma_start(out=wq_sb, in_=wq_d.ap())
        nc.sync.dma_start(out=wk_sb, in_=wk_d.ap())
        xt = [xts.tile([P, T], BF16, tag="xt", name=f"xt{i}") for i in range(NCB)]
        for cb in range(NCB):
            nc.sync.dma_start(out=xt[cb], in_=x_d.ap()[cb * P : (cb + 1) * P, :])
        wv_sb = bigs.tile([P, NCB, FPC], BF16, tag="wv")
        wp_sb = bigs.tile([P, 2, C], BF16, tag="wp")
        nc.sync.dma_start(out=wv_sb, in_=wv_d.ap())
        nc.sync.dma_start(out=wp_sb, in_=wp_d.ap())
        bq_sb = consts.tile([P, 2], F32, tag="bq")
        bk_sb = consts.tile([P, 2], F32, tag="bk")
        bv_sb = consts.tile([1, FPC], BF16, tag="bv")
        triu = consts.tile([P, P], BF16, tag="triu")
        ones1 = consts.tile([1, P], BF16, tag="ones1")
        for t_, d_ in ((bq_sb, bq_d), (bk_sb, bk_d), (bv_sb, bv_d),
                       (triu, triu_d), (ones1, ones1_d)):
            nc.sync.dma_start(out=t_, in_=d_.ap())

        qt = [bigs.tile([P, T], BF16, tag=f"qt{i}", name=f"qt{i}") for i in range(2)]
        kt = [bigs.tile([P, T], BF16, tag=f"kt{i}", name=f"kt{i}") for i in range(2)]
        yt = [bigs.tile([P, T], BF16, tag=f"yt{i}", name=f"yt{i}") for i in range(2)]
        # V layout [P(t-rows), NT, pair, 65*2]: per pair, head-A cols 0:64 +
        # ones col 64; head-B cols 65:129 + ones col 129.  The ones column
        # makes the PV matmul emit the softmax denominator in PSUM row 64.
        v_sb = bigs.tile([P, NT, 2, 130], BF16, tag="v")
        nc.gpsimd.memset(v_sb[:, :, :, 64], 1.0)
        nc.gpsimd.memset(v_sb[:, :, :, 129], 1.0)

        def qkv_chunk(qc):
            """QK feature tiles for query window qc + V tiles for the same
            token window.  qc==0 runs cb-outer so matmuls start as soon as
            each x^T tile lands."""
            cs = slice(qc * QCW, (qc + 1) * QCW)
            groups = [(wq_sb, qt, bq_sb, 0), (wq_sb, qt, bq_sb, 1),
                      (wk_sb, kt, bk_sb, 0), (wk_sb, kt, bk_sb, 1)]
            pss = []
            if qc == 0:
                for gi in range(4):
                    pss.append(psum.tile([P, QCW], F32, tag="ps", name=f"qk{gi}"))
                for cb in range(NCB):
                    for gi, (wsb, _, _, ft) in enumerate(groups):
                        nc.tensor.matmul(
                            pss[gi], wsb[:, cb, ft * P : (ft + 1) * P],
                            xt[cb][:, cs], start=(cb == 0), stop=(cb == NCB - 1),
                        )
            else:
                for gi, (wsb, _, _, ft) in enumerate(groups):
                    ps = psum.tile([P, QCW], F32, tag="ps", name=f"qk{gi}")
                    pss.append(ps)
                    for cb in range(NCB):
                        nc.tensor.matmul(
                            ps, wsb[:, cb, ft * P : (ft + 1) * P],
                            xt[cb][:, cs], start=(cb == 0), stop=(cb == NCB - 1),
                        )
            for gi, (_, dst, bsb, ft) in enumerate(groups):
                nc.vector.tensor_scalar_add(dst[ft][:, cs], pss[gi], bsb[:, ft : ft + 1])
            for t in range(4 * qc, 4 * qc + 4):
                ps = psum.tile([P, FPC], F32, tag="ps", name="v")
                for cb in range(NCB):
                    nc.tensor.matmul(
                        ps, xt[cb][:, t * P : (t + 1) * P], wv_sb[:, cb, :],
                        start=(cb == 0), stop=False,
                    )
                nc.tensor.matmul(ps, ones1, bv_sb, start=False, stop=True)
                nc.vector.tensor_copy(
                    out=v_sb[:, t].rearrange("p a (h w) -> p a h w", w=65)[:, :, :, 0:64],
                    in_=ps.rearrange("p (a h w) -> p a h w", a=2, w=64),
                )

        def attn_pair(qc, pair):
            cs = slice(qc * QCW, (qc + 1) * QCW)
            cs0 = qc * QCW
            nki = 4 * (qc + 1)
            yA = psum.tile([P, QCW], F32, tag="ps", name="yA")
            yB = psum.tile([P, QCW], F32, tag="ps", name="yB")
            pend = []  # (ki, lo, eA, eB) with S+exp emitted, PV not yet

            def emit_s(ki):
                m = ki - 4 * qc
                lo = max(m, 0) * P
                ks = slice(ki * P, (ki + 1) * P)
                stA = psum.tile([P, QCW], F32, tag="ps", name="stA")
                stB = psum.tile([P, QCW], F32, tag="ps", name="stB")
                nc.tensor.matmul(
                    stA[:, lo:], kt[pair][0:64, ks],
                    qt[pair][0:64, cs0 + lo : cs0 + QCW], start=True, stop=True,
                )
                nc.tensor.matmul(
                    stB[:, lo:], kt[pair][64:P, ks],
                    qt[pair][64:P, cs0 + lo : cs0 + QCW], start=True, stop=True,
                    tile_position=(64, 0),
                )
                eA = exps.tile([P, QCW], BF16, tag="exp", name="eA")
                eB = exps.tile([P, QCW], BF16, tag="exp", name="eB")
                nc.scalar.activation(eA[:, lo:], stA[:, lo:], EXP, scale=0.125)
                nc.scalar.activation(eB[:, lo:], stB[:, lo:], EXP, scale=0.125)
                if m >= 0:  # diagonal 128-block: causal triangle mask
                    ds_ = slice(m * P, (m + 1) * P)
                    nc.vector.tensor_mul(eA[:, ds_], eA[:, ds_], triu)
                    nc.vector.tensor_mul(eB[:, ds_], eB[:, ds_], triu)
                pend.append((ki, lo, eA, eB))

            def emit_pv():
                ki, lo, eA, eB = pend.pop(0)
                st, sp = ki == 0, ki == nki - 1
                nc.tensor.matmul(
                    yA[0:65, lo:], v_sb[:, ki, pair, 0:65], eA[:, lo:],
                    start=st, stop=sp,
                )
                nc.tensor.matmul(
                    yB[0:65, lo:], v_sb[:, ki, pair, 65:130], eB[:, lo:],
                    start=st, stop=sp,
                )

            emit_s(0)
            for ki in range(nki):
                if ki + 1 < nki:
                    emit_s(ki + 1)
                emit_pv()

            # normalize: 1/sum from PSUM row 64, broadcast to 64 partitions,
            # multiply straight out of PSUM into the bf16 y^T tile.
            for half, yps in ((0, yA), (1, yB)):
                rec = recs.tile([1, QCW], F32, tag="rec")
                nc.vector.reciprocal(rec, yps[64:65, :])
                recb = recbs.tile([64, QCW], F32, tag="recb")
                nc.gpsimd.dma_start(
                    out=recb, in_=rec[0:1, None, :].broadcast_to([1, 64, QCW])
                )
                nc.vector.tensor_mul(
                    yt[pair][half * 64 : half * 64 + 64, cs], yps[0:64, :], recb
                )

        def proj_chunk(qc):
            for t in range(4 * qc, 4 * qc + 4):
                ts = slice(t * P, (t + 1) * P)
                for nch in range(2):
                    ps = psum.tile([P, QCW], F32, tag="ps", name="proj")
                    for fb in range(2):
                        nc.tensor.matmul(
                            ps, yt[fb][:, ts],
                            wp_sb[:, fb, nch * QCW : (nch + 1) * QCW],
                            start=(fb == 0), stop=(fb == 1),
                        )
                    ost = stage.tile([P, QCW], F32, tag="stage")
                    if nch == 0:
                        nc.vector.tensor_copy(out=ost, in_=ps)
                    else:
                        nc.scalar.copy(out=ost, in_=ps)
                    nc.sync.dma_start(
                        out=out_d.ap()[ts, nch * QCW : (nch + 1) * QCW], in_=ost
                    )

        qkv_chunk(0)
        for qc in range(NQC):
            attn_pair(qc, 0)
            attn_pair(qc, 1)
            if qc + 1 < NQC:
                qkv_chunk(qc + 1)
            proj_chunk(qc)

    nc.compile()
    return nc


_NC_CACHE: dict = {}
LAST_RESULT = None


def kernel(x, w_attn, b_attn, w_proj, b_proj):
    global LAST_RESULT
    x = np.asarray(x, np.float32)
    w_attn = np.asarray(w_attn, np.float32)
    b_attn = np.asarray(b_attn, np.float32)
    w_proj = np.asarray(w_proj, np.float32)
    b_proj = np.asarray(b_proj, np.float32)

    if "nc" not in _NC_CACHE:
        _NC_CACHE["nc"] = build_nc()
    nc = _NC_CACHE["nc"]

    triu = np.triu(np.ones((P, P), np.float32)).astype(BF)
    ones1 = np.ones((1, P), BF)
    xT = [np.ascontiguousarray(x[b].T).astype(BF) for b in range(B)]

    in_maps = []
    for core in range(8):
        b, g = core // 4, core % 4
        f0 = g * FPC

        def wsect(off):
            w = w_attn[:, off + f0 : off + f0 + FPC]
            return np.ascontiguousarray(
                w.reshape(NCB, P, FPC).transpose(1, 0, 2)
            ).astype(BF)

        in_maps.append(
            {
                "x": xT[b],
                "wq": wsect(0),
                "wk": wsect(C),
                "wv": wsect(2 * C),
                "wp": np.ascontiguousarray(
                    w_proj[f0 : f0 + FPC, :].reshape(2, P, C).transpose(1, 0, 2)
                ).astype(BF),
                "bq": np.ascontiguousarray(
                    b_attn[f0 : f0 + FPC].reshape(2, P).T
                ).astype(np.float32),
                "bk": np.ascontiguousarray(
                    b_attn[C + f0 : C + f0 + FPC].reshape(2, P).T
                ).astype(np.float32),
                "bv": b_attn[2 * C + f0 : 2 * C + f0 + FPC].reshape(1, FPC).astype(BF),
                "triu": triu,
                "ones1": ones1,
            }
        )

    trace = bool(os.environ.get("BASS_TRACE"))
    res = run_bass_kernel_spmd(
        nc,
        in_maps,
        core_ids=list(range(8)),
        trace=trace,
        tmpdir=os.environ.get("KERNEL_TRACE_DIR") or None,
    )
    LAST_RESULT = res

    y = np.empty((B, T, C), np.float32)
    for b in range(B):
        acc = res.results[4 * b]["out"].astype(np.float32).copy()
        for g in range(1, 4):
            acc += res.results[4 * b + g]["out"]
        y[b] = acc + b_proj[None, :]
    return y


# revision 6
# speedup vs baseline: 1.6993x; 1.1919x over previous
"""Causal self-attention (B=2, T=2048, C=1024, NH=16) on 8 trn2 NeuronCores.

Sharding: core c handles batch b = c//4 and head group g = c%4 (4 heads,
256 features). Each core computes q/k/v for its heads, causal attention in
S^T layout (keys on partitions, queries on the free dim), and a partial
output projection  y_heads @ w_proj[head_rows, :].  The host sums the four
partial projections per batch and adds b_proj.

v3 design notes:
  - All matmul operands bf16 (PSUM accumulation fp32); x is transposed and
    all layouts pre-arranged on the host so every DMA is contiguous.
  - V tiles carry a 64-wide block of ones per head, so the PV matmul drops
    the softmax denominator onto PSUM partitions 64:128 (64 copies of it):
    the reciprocal runs as a parallel [64,512] DVE op and feeds the
    normalizing multiply directly -- no single-partition serial ops, no
    partition-broadcast DMA.
  - Per k-tile the two packed heads' scores land in one 2-bank PSUM tile
    and are exponentiated by a single wide scalar-engine instruction
    (scale=1/8 folded in). Causal masks multiply on the Pool engine.
  - Emission interleaves QKV(qc+1)/proj(qc-1) work-units into the attention
    ki-stream, and PV trails exp by PVLAG steps, so the in-order PE queue
    never parks behind a scalar-engine dependency (keeps the HAM clock
    gate released at 2.4 GHz).
"""

import os
import sys
from collections import deque

import numpy as np
import ml_dtypes

for _p in ("/opt/trn_rl_repo", "/root/.axon_site/_ro/trn_rl_repo"):
    if _p not in sys.path and os.path.isdir(_p):
        sys.path.append(_p)

import concourse.bass as bass  # noqa: E402
import concourse.tile as tile  # noqa: E402
from concourse import bacc, mybir  # noqa: E402
from concourse.bass_utils import run_bass_kernel_spmd  # noqa: E402

P = 128
B, T, C = 2, 2048, 1024
NH, HD = 16, 64
HPC = 4  # heads per core
FPC = HPC * HD  # features per core (256)
QCW = 512  # query-chunk width (PSUM bank = 512 fp32)
NQC = T // QCW
NT = T // P
NCB = C // P
PVLAG = 2  # ki-steps PV trails exp by
F32 = mybir.dt.float32
BF16 = mybir.dt.bfloat16
BF = ml_dtypes.bfloat16
EXP = mybir.ActivationFunctionType.Exp


def build_nc():
    nc = bacc.Bacc("TRN2", target_bir_lowering=False, debug=False)

    x_d = nc.dram_tensor("x", [C, T], BF16, kind="ExternalInput")  # x^T
    wq_d = nc.dram_tensor("wq", [P, NCB, FPC], BF16, kind="ExternalInput")
    wk_d = nc.dram_tensor("wk", [P, NCB, FPC], BF16, kind="ExternalInput")
    wv_d = nc.dram_tensor("wv", [P, NCB, FPC], BF16, kind="ExternalInput")
    wp_d = nc.dram_tensor("wp", [P, 2, C], BF16, kind="ExternalInput")
    bq_d = nc.dram_tensor("bq", [P, 2], F32, kind="ExternalInput")
    bk_d = nc.dram_tensor("bk", [P, 2], F32, kind="ExternalInput")
    bv_d = nc.dram_tensor("bv", [1, FPC], BF16, kind="ExternalInput")
    triu_d = nc.dram_tensor("triu", [P, P], BF16, kind="ExternalInput")
    ones1_d = nc.dram_tensor("ones1", [1, P], BF16, kind="ExternalInput")
    out_d = nc.dram_tensor("out", [T, C], F32, kind="ExternalOutput")

    from contextlib import ExitStack

    with tile.TileContext(nc) as tc, ExitStack() as ctx:
        consts = ctx.enter_context(tc.tile_pool(name="consts", bufs=1))
        bigs = ctx.enter_context(tc.tile_pool(name="bigs", bufs=1))
        xts = ctx.enter_context(tc.tile_pool(name="xts", bufs=NCB))
        exps = ctx.enter_context(tc.tile_pool(name="exps", bufs=5))
        recs = ctx.enter_context(tc.tile_pool(name="recs", bufs=2))
        stage = ctx.enter_context(tc.tile_pool(name="stage", bufs=3))
        psum = ctx.enter_context(tc.tile_pool(name="psum", bufs=2, space="PSUM"))

        # ---- weights / consts into SBUF (all DMAs fully contiguous) ----
        wq_sb = bigs.tile([P, NCB, FPC], BF16, tag="wq")
        wk_sb = bigs.tile([P, NCB, FPC], BF16, tag="wk")
        nc.sync.dma_start(out=wq_sb, in_=wq_d.ap())
        nc.sync.dma_start(out=wk_sb, in_=wk_d.ap())
        xt = [xts.tile([P, T], BF16, tag="xt", name=f"xt{i}") for i in range(NCB)]
        for cb in range(NCB):
            nc.sync.dma_start(out=xt[cb], in_=x_d.ap()[cb * P : (cb + 1) * P, :])
        wv_sb = bigs.tile([P, NCB, FPC], BF16, tag="wv")
        wp_sb = bigs.tile([P, 2, C], BF16, tag="wp")
        nc.sync.dma_start(out=wv_sb, in_=wv_d.ap())
        nc.sync.dma_start(out=wp_sb, in_=wp_d.ap())
        bq_sb = consts.tile([P, 2], F32, tag="bq")
        bk_sb = consts.tile([P, 2], F32, tag="bk")
        bv_sb = consts.tile([1, FPC], BF16, tag="bv")
        triu = consts.tile([P, P], BF16, tag="triu")
        ones1 = consts.tile([1, P], BF16, tag="ones1")
        for t_, d_ in ((bq_sb, bq_d), (bk_sb, bk_d), (bv_sb, bv_d),
                       (triu, triu_d), (ones1, ones1_d)):
            nc.sync.dma_start(out=t_, in_=d_.ap())

        qt = [bigs.tile([P, T], BF16, tag=f"qt{i}", name=f"qt{i}") for i in range(2)]
        kt = [bigs.tile([P, T], BF16, tag=f"kt{i}", name=f"kt{i}") for i in range(2)]
        yt = [bigs.tile([P, T], BF16, tag=f"yt{i}", name=f"yt{i}") for i in range(2)]
        # V layout [P(t-rows), NT, pair, head, 128]: per head cols 0:64 hold
        # v, cols 64:128 hold ones.  PV with this 128-wide stationary tile
        # puts y on PSUM rows 0:64 and 64 copies of the softmax denominator
        # on rows 64:128 (so 1/sum is a parallel 64-partition DVE op).
        v_sb = bigs.tile([P, NT, 2, 2, P], BF16, tag="v")
        nc.gpsimd.memset(
            v_sb.rearrange("p t a h w -> p (t a h) w")[:, :, 64:P], 1.0
        )

        # ---------- work-unit emitters ----------
        def qk_group(qc, wsb, bsb, dst, ft):
            cs = slice(qc * QCW, (qc + 1) * QCW)
            ps = psum.tile([P, QCW], F32, tag="mm", name="qk", bufs=2)
            for cb in range(NCB):
                nc.tensor.matmul(
                    ps, wsb[:, cb, ft * P : (ft + 1) * P], xt[cb][:, cs],
                    start=(cb == 0), stop=(cb == NCB - 1),
                )
            nc.vector.tensor_scalar_add(dst[ft][:, cs], ps, bsb[:, ft : ft + 1])

        def v_unit(t):
            ps = psum.tile([P, FPC], F32, tag="mm", name="v", bufs=2)
            for cb in range(NCB):
                nc.tensor.matmul(
                    ps, xt[cb][:, t * P : (t + 1) * P], wv_sb[:, cb, :],
                    start=(cb == 0), stop=False,
                )
            nc.tensor.matmul(ps, ones1, bv_sb, start=False, stop=True)
            nc.vector.tensor_copy(
                out=v_sb[:, t, :, :, 0:64].rearrange("p a h w -> p (a h) w"),
                in_=ps.rearrange("p (a h w) -> p (a h) w", a=2, w=64),
            )

        def proj_unit(t, nch):
            ts = slice(t * P, (t + 1) * P)
            ps = psum.tile([P, QCW], F32, tag="mm", name="proj", bufs=2)
            for fb in range(2):
                nc.tensor.matmul(
                    ps, yt[fb][:, ts], wp_sb[:, fb, nch * QCW : (nch + 1) * QCW],
                    start=(fb == 0), stop=(fb == 1),
                )
            ost = stage.tile([P, QCW], F32, tag="stage")
            nc.vector.tensor_copy(out=ost, in_=ps)
            nc.sync.dma_start(
                out=out_d.ap()[ts, nch * QCW : (nch + 1) * QCW], in_=ost
            )

        def qkv_units(qc):
            for gi, (wsb, bsb, dst) in enumerate(
                ((wq_sb, bq_sb, qt), (wk_sb, bk_sb, kt))
            ):
                for ft in range(2):
                    yield lambda w=wsb, b=bsb, d=dst, f=ft: qk_group(qc, w, b, d, f)
            for t in range(4 * qc, 4 * qc + 4):
                yield lambda t_=t: v_unit(t_)

        def proj_units(qc):
            for t in range(4 * qc, 4 * qc + 4):
                for nch in range(2):
                    yield lambda t_=t, n_=nch: proj_unit(t_, n_)

        # ---------- attention ----------
        def attn(qc, inj):
            cs = slice(qc * QCW, (qc + 1) * QCW)
            cs0 = qc * QCW
            nki = 4 * (qc + 1)
            for pair in range(2):
                yA = psum.tile([P, QCW], F32, tag="y", name="yA", bufs=2)
                yB = psum.tile([P, QCW], F32, tag="y", name="yB", bufs=2)
                pend = deque()

                def emit_s(ki):
                    m = ki - 4 * qc
                    lo = max(m, 0) * P
                    ks = slice(ki * P, (ki + 1) * P)
                    st = psum.tile([P, 2, QCW], F32, tag="st", name="st", bufs=2)
                    nc.tensor.matmul(
                        st[:, 0, lo:], kt[pair][0:64, ks],
                        qt[pair][0:64, cs0 + lo : cs0 + QCW], start=True, stop=True,
                    )
                    nc.tensor.matmul(
                        st[:, 1, lo:], kt[pair][64:P, ks],
                        qt[pair][64:P, cs0 + lo : cs0 + QCW], start=True, stop=True,
                        tile_position=(64, 0),
                    )
                    e = exps.tile([P, 2, QCW], BF16, tag="exp", name="e")
                    nc.scalar.activation(e[:, :, lo:], st[:, :, lo:], EXP, scale=0.125)
                    if m >= 0:  # diagonal 128-block: causal triangle mask
                        ds_ = slice(m * P, (m + 1) * P)
                        nc.gpsimd.tensor_mul(e[:, 0, ds_], e[:, 0, ds_], triu)
                        nc.gpsimd.tensor_mul(e[:, 1, ds_], e[:, 1, ds_], triu)
                    pend.append((ki, lo, e))

                def emit_pv():
                    ki, lo, e = pend.popleft()
                    st_, sp = ki == 0, ki == nki - 1
                    nc.tensor.matmul(
                        yA[:, lo:], v_sb[:, ki, pair, 0], e[:, 0, lo:],
                        start=st_, stop=sp,
                    )
                    nc.tensor.matmul(
                        yB[:, lo:], v_sb[:, ki, pair, 1], e[:, 1, lo:],
                        start=st_, stop=sp,
                    )

                for ki in range(nki):
                    emit_s(ki)
                    if len(pend) > PVLAG:
                        emit_pv()
                    if inj:
                        inj.popleft()()
                while pend:
                    emit_pv()
                    if inj:
                        inj.popleft()()

                # normalize: parallel [64,512] reciprocal of the denominator
                # rows, multiply straight out of PSUM into the bf16 y^T tile.
                for half, yps in ((0, yA), (1, yB)):
                    rec = recs.tile([64, QCW], F32, tag="rec")
                    nc.vector.reciprocal(rec, yps[64:P, :])
                    nc.vector.tensor_mul(
                        yt[pair][half * 64 : half * 64 + 64, cs], yps[0:64, :], rec
                    )

        # ---------- program ----------
        # qc=0 QKV directly (DMA-paced ramp: matmuls start as x^T tiles land)
        for unit in qkv_units(0):
            unit()
        inj = deque()
        for qc in range(NQC):
            if qc + 1 < NQC:
                inj.extend(qkv_units(qc + 1))
            attn(qc, inj)
            inj.extend(proj_units(qc))
        while inj:
            inj.popleft()()

    nc.compile()
    return nc


_NC_CACHE: dict = {}
LAST_RESULT = None


def kernel(x, w_attn, b_attn, w_proj, b_proj):
    global LAST_RESULT
    x = np.asarray(x, np.float32)
    w_attn = np.asarray(w_attn, np.float32)
    b_attn = np.asarray(b_attn, np.float32)
    w_proj = np.asarray(w_proj, np.float32)
    b_proj = np.asarray(b_proj, np.float32)

    if "nc" not in _NC_CACHE:
        _NC_CACHE["nc"] = build_nc()
    nc = _NC_CACHE["nc"]

    triu = np.triu(np.ones((P, P), np.float32)).astype(BF)
    ones1 = np.ones((1, P), BF)
    xT = [np.ascontiguousarray(x[b].T).astype(BF) for b in range(B)]

    in_maps = []
    for core in range(8):
        b, g = core // 4, core % 4
        f0 = g * FPC

        def wsect(off):
            w = w_attn[:, off + f0 : off + f0 + FPC]
            return np.ascontiguousarray(
                w.reshape(NCB, P, FPC).transpose(1, 0, 2)
            ).astype(BF)

        in_maps.append(
            {
                "x": xT[b],
                "wq": wsect(0),
                "wk": wsect(C),
                "wv": wsect(2 * C),
                "wp": np.ascontiguousarray(
                    w_proj[f0 : f0 + FPC, :].reshape(2, P, C).transpose(1, 0, 2)
                ).astype(BF),
                "bq": np.ascontiguousarray(
                    b_attn[f0 : f0 + FPC].reshape(2, P).T
                ).astype(np.float32),
                "bk": np.ascontiguousarray(
                    b_attn[C + f0 : C + f0 + FPC].reshape(2, P).T
                ).astype(np.float32),
                "bv": b_attn[2 * C + f0 : 2 * C + f0 + FPC].reshape(1, FPC).astype(BF),
                "triu": triu,
                "ones1": ones1,
            }
        )

    trace = bool(os.environ.get("BASS_TRACE"))
    res = run_bass_kernel_spmd(
        nc,
        in_maps,
        core_ids=list(range(8)),
        trace=trace,
        tmpdir=os.environ.get("KERNEL_TRACE_DIR") or None,
    )
    LAST_RESULT = res

    y = np.empty((B, T, C), np.float32)
    for b in range(B):
        acc = res.results[4 * b]["out"].astype(np.float32).copy()
        for g in range(1, 4):
            acc += res.results[4 * b + g]["out"]
        y[b] = acc + b_proj[None, :]
    return y


# revision 8
# speedup vs baseline: 2.1258x; 1.2510x over previous
"""Causal self-attention (B=2, T=2048, C=1024, NH=16) on 8 trn2 NeuronCores.

Sharding: core c handles batch b = c//4 and head group g = c%4 (4 heads,
256 features). Each core computes q/k/v for its heads, causal attention in
S^T layout (keys on partitions, queries on the free dim), and a partial
output projection  y_heads @ w_proj[head_rows, :].  The host sums the four
partial projections per batch and adds b_proj.

v3 design notes:
  - All matmul operands bf16 (PSUM accumulation fp32); x is transposed and
    all layouts pre-arranged on the host so every DMA is contiguous.
  - V tiles carry a 64-wide block of ones per head, so the PV matmul drops
    the softmax denominator onto PSUM partitions 64:128 (64 copies of it):
    the reciprocal runs as a parallel [64,512] DVE op and feeds the
    normalizing multiply directly -- no single-partition serial ops, no
    partition-broadcast DMA.
  - Per k-tile the two packed heads' scores land in one 2-bank PSUM tile
    and are exponentiated by a single wide scalar-engine instruction
    (scale=1/8 folded in). Causal masks multiply on the Pool engine.
  - Emission interleaves QKV(qc+1)/proj(qc-1) work-units into the attention
    ki-stream, and PV trails exp by PVLAG steps, so the in-order PE queue
    never parks behind a scalar-engine dependency (keeps the HAM clock
    gate released at 2.4 GHz).
"""

import os
import sys
from collections import deque

import numpy as np
import ml_dtypes

for _p in ("/opt/trn_rl_repo", "/root/.axon_site/_ro/trn_rl_repo"):
    if _p not in sys.path and os.path.isdir(_p):
        sys.path.append(_p)

import concourse.bass as bass  # noqa: E402
import concourse.tile as tile  # noqa: E402
from concourse import bacc, mybir  # noqa: E402
from concourse.bass_utils import run_bass_kernel_spmd  # noqa: E402

P = 128
B, T, C = 2, 2048, 1024
NH, HD = 16, 64
HPC = 4  # heads per core
FPC = HPC * HD  # features per core (256)
QCW = 512  # query-chunk width (PSUM bank = 512 fp32)
NQC = T // QCW
NT = T // P
NCB = C // P
PVLAG = 2  # ki-steps PV trails exp by
F32 = mybir.dt.float32
BF16 = mybir.dt.bfloat16
BF = ml_dtypes.bfloat16
EXP = mybir.ActivationFunctionType.Exp


def build_nc():
    nc = bacc.Bacc("TRN2", target_bir_lowering=False, debug=False)

    x_d = nc.dram_tensor("x", [C, T], BF16, kind="ExternalInput")  # x^T
    wq_d = nc.dram_tensor("wq", [P, NCB, FPC], BF16, kind="ExternalInput")
    wk_d = nc.dram_tensor("wk", [P, NCB, FPC], BF16, kind="ExternalInput")
    wv_d = nc.dram_tensor("wv", [P, NCB, FPC], BF16, kind="ExternalInput")
    wp_d = nc.dram_tensor("wp", [P, 2, C], BF16, kind="ExternalInput")
    bq_d = nc.dram_tensor("bq", [P, 2], F32, kind="ExternalInput")
    bk_d = nc.dram_tensor("bk", [P, 2], F32, kind="ExternalInput")
    bv_d = nc.dram_tensor("bv", [1, FPC], BF16, kind="ExternalInput")
    triu_d = nc.dram_tensor("triu", [P, P], BF16, kind="ExternalInput")
    ones1_d = nc.dram_tensor("ones1", [1, P], BF16, kind="ExternalInput")
    out_d = nc.dram_tensor("out", [T, C], F32, kind="ExternalOutput")

    from contextlib import ExitStack

    with tile.TileContext(nc) as tc, ExitStack() as ctx:
        consts = ctx.enter_context(tc.tile_pool(name="consts", bufs=1))
        bigs = ctx.enter_context(tc.tile_pool(name="bigs", bufs=1))
        xts = ctx.enter_context(tc.tile_pool(name="xts", bufs=NCB))
        exps = ctx.enter_context(tc.tile_pool(name="exps", bufs=5))
        recs = ctx.enter_context(tc.tile_pool(name="recs", bufs=2))
        stage = ctx.enter_context(tc.tile_pool(name="stage", bufs=3))
        psum = ctx.enter_context(tc.tile_pool(name="psum", bufs=2, space="PSUM"))

        # ---- weights / consts into SBUF (all DMAs fully contiguous) ----
        wq_sb = bigs.tile([P, NCB, FPC], BF16, tag="wq")
        wk_sb = bigs.tile([P, NCB, FPC], BF16, tag="wk")
        nc.sync.dma_start(out=wq_sb, in_=wq_d.ap())
        nc.sync.dma_start(out=wk_sb, in_=wk_d.ap())
        xt = [xts.tile([P, T], BF16, tag="xt", name=f"xt{i}") for i in range(NCB)]
        for cb in range(NCB):
            nc.sync.dma_start(out=xt[cb], in_=x_d.ap()[cb * P : (cb + 1) * P, :])
        wv_sb = bigs.tile([P, NCB, FPC], BF16, tag="wv")
        wp_sb = bigs.tile([P, 2, C], BF16, tag="wp")
        nc.sync.dma_start(out=wv_sb, in_=wv_d.ap())
        nc.sync.dma_start(out=wp_sb, in_=wp_d.ap())
        bq_sb = consts.tile([P, 2], F32, tag="bq")
        bk_sb = consts.tile([P, 2], F32, tag="bk")
        bv_sb = consts.tile([1, FPC], BF16, tag="bv")
        triu = consts.tile([P, P], BF16, tag="triu")
        ones1 = consts.tile([1, P], BF16, tag="ones1")
        for t_, d_ in ((bq_sb, bq_d), (bk_sb, bk_d), (bv_sb, bv_d),
                       (triu, triu_d), (ones1, ones1_d)):
            nc.sync.dma_start(out=t_, in_=d_.ap())

        qt = [bigs.tile([P, T], BF16, tag=f"qt{i}", name=f"qt{i}") for i in range(2)]
        kt = [bigs.tile([P, T], BF16, tag=f"kt{i}", name=f"kt{i}") for i in range(2)]
        yt = [bigs.tile([P, T], BF16, tag=f"yt{i}", name=f"yt{i}") for i in range(2)]
        # V layout [P(t-rows), NT, pair, head, 128]: per head cols 0:64 hold
        # v, cols 64:128 hold ones.  PV with this 128-wide stationary tile
        # puts y on PSUM rows 0:64 and 64 copies of the softmax denominator
        # on rows 64:128 (so 1/sum is a parallel 64-partition DVE op).
        v_sb = bigs.tile([P, NT, 2, 2, P], BF16, tag="v")
        nc.gpsimd.memset(
            v_sb.rearrange("p t a h w -> p (t a h) w")[:, :, 64:P], 1.0
        )

        # ---------- work-unit emitters ----------
        def qk_group(qc, wsb, bsb, dst, ft):
            cs = slice(qc * QCW, (qc + 1) * QCW)
            ps = psum.tile([P, QCW], F32, tag="mm", name="qk", bufs=2)
            for cb in range(NCB):
                nc.tensor.matmul(
                    ps, wsb[:, cb, ft * P : (ft + 1) * P], xt[cb][:, cs],
                    start=(cb == 0), stop=(cb == NCB - 1),
                )
            nc.vector.tensor_scalar_add(dst[ft][:, cs], ps, bsb[:, ft : ft + 1])

        def v_unit(t):
            ps = psum.tile([P, FPC], F32, tag="mm", name="v", bufs=2)
            for cb in range(NCB):
                nc.tensor.matmul(
                    ps, xt[cb][:, t * P : (t + 1) * P], wv_sb[:, cb, :],
                    start=(cb == 0), stop=False,
                )
            nc.tensor.matmul(ps, ones1, bv_sb, start=False, stop=True)
            nc.vector.tensor_copy(
                out=v_sb[:, t, :, :, 0:64].rearrange("p a h w -> p (a h) w"),
                in_=ps.rearrange("p (a h w) -> p (a h) w", a=2, w=64),
            )

        def proj_unit(t, nch):
            ts = slice(t * P, (t + 1) * P)
            ps = psum.tile([P, QCW], F32, tag="mm", name="proj", bufs=2)
            for fb in range(2):
                nc.tensor.matmul(
                    ps, yt[fb][:, ts], wp_sb[:, fb, nch * QCW : (nch + 1) * QCW],
                    start=(fb == 0), stop=(fb == 1),
                )
            ost = stage.tile([P, QCW], F32, tag="stage")
            nc.vector.tensor_copy(out=ost, in_=ps)
            nc.sync.dma_start(
                out=out_d.ap()[ts, nch * QCW : (nch + 1) * QCW], in_=ost
            )

        def qkv_units(qc):
            for gi, (wsb, bsb, dst) in enumerate(
                ((wq_sb, bq_sb, qt), (wk_sb, bk_sb, kt))
            ):
                for ft in range(2):
                    yield lambda w=wsb, b=bsb, d=dst, f=ft: qk_group(qc, w, b, d, f)
            for t in range(4 * qc, 4 * qc + 4):
                yield lambda t_=t: v_unit(t_)

        def proj_units(qc):
            for t in range(4 * qc, 4 * qc + 4):
                for nch in range(2):
                    yield lambda t_=t, n_=nch: proj_unit(t_, n_)

        # ---------- attention ----------
        def attn(qc, inj):
            cs = slice(qc * QCW, (qc + 1) * QCW)
            cs0 = qc * QCW
            nki = 4 * (qc + 1)
            for pair in range(2):
                yA = psum.tile([P, QCW], F32, tag="y", name="yA", bufs=2)
                yB = psum.tile([P, QCW], F32, tag="y", name="yB", bufs=2)
                pend = deque()

                def emit_s(ki):
                    m = ki - 4 * qc
                    lo = max(m, 0) * P
                    ks = slice(ki * P, (ki + 1) * P)
                    st = psum.tile([P, 2, QCW], F32, tag="st", name="st", bufs=2)
                    nc.tensor.matmul(
                        st[:, 0, lo:], kt[pair][0:64, ks],
                        qt[pair][0:64, cs0 + lo : cs0 + QCW], start=True, stop=True,
                    )
                    nc.tensor.matmul(
                        st[:, 1, lo:], kt[pair][64:P, ks],
                        qt[pair][64:P, cs0 + lo : cs0 + QCW], start=True, stop=True,
                        tile_position=(64, 0),
                    )
                    e = exps.tile([P, 2, QCW], BF16, tag="exp", name="e")
                    nc.scalar.activation(e[:, :, lo:], st[:, :, lo:], EXP, scale=0.125)
                    if m >= 0:  # diagonal 128-block: causal triangle mask
                        ds_ = slice(m * P, (m + 1) * P)
                        nc.gpsimd.tensor_mul(e[:, 0, ds_], e[:, 0, ds_], triu)
                        nc.gpsimd.tensor_mul(e[:, 1, ds_], e[:, 1, ds_], triu)
                    pend.append((ki, lo, e))

                def emit_pv():
                    ki, lo, e = pend.popleft()
                    st_, sp = ki == 0, ki == nki - 1
                    nc.tensor.matmul(
                        yA[:, lo:], v_sb[:, ki, pair, 0], e[:, 0, lo:],
                        start=st_, stop=sp,
                    )
                    nc.tensor.matmul(
                        yB[:, lo:], v_sb[:, ki, pair, 1], e[:, 1, lo:],
                        start=st_, stop=sp,
                    )

                for ki in range(nki):
                    emit_s(ki)
                    if len(pend) > PVLAG:
                        emit_pv()
                    if inj:
                        inj.popleft()()
                while pend:
                    emit_pv()
                    if inj:
                        inj.popleft()()

                # normalize: parallel [64,512] reciprocal of the denominator
                # rows, multiply straight out of PSUM into the bf16 y^T tile.
                for half, yps in ((0, yA), (1, yB)):
                    ssum = recs.tile([64, QCW], F32, tag="ssum")
                    nc.vector.tensor_copy(out=ssum, in_=yps[64:P, :])
                    rec = recs.tile([64, QCW], F32, tag="rec")
                    nc.vector.reciprocal_approx_fast(out=rec, in_=ssum)
                    nc.vector.tensor_mul(
                        yt[pair][half * 64 : half * 64 + 64, cs], yps[0:64, :], rec
                    )

        # ---------- program ----------
        # qc=0 QKV directly (DMA-paced ramp: matmuls start as x^T tiles land)
        for unit in qkv_units(0):
            unit()
        inj = deque()
        for qc in range(NQC):
            if qc + 1 < NQC:
                inj.extend(qkv_units(qc + 1))
            attn(qc, inj)
            inj.extend(proj_units(qc))
        while inj:
            inj.popleft()()

    nc.compile()
    return nc


_NC_CACHE: dict = {}
LAST_RESULT = None


def kernel(x, w_attn, b_attn, w_proj, b_proj):
    global LAST_RESULT
    x = np.asarray(x, np.float32)
    w_attn = np.asarray(w_attn, np.float32)
    b_attn = np.asarray(b_attn, np.float32)
    w_proj = np.asarray(w_proj, np.float32)
    b_proj = np.asarray(b_proj, np.float32)

    if "nc" not in _NC_CACHE:
        _NC_CACHE["nc"] = build_nc()
    nc = _NC_CACHE["nc"]

    triu = np.triu(np.ones((P, P), np.float32)).astype(BF)
    ones1 = np.ones((1, P), BF)
    xT = [np.ascontiguousarray(x[b].T).astype(BF) for b in range(B)]

    in_maps = []
    for core in range(8):
        b, g = core // 4, core % 4
        f0 = g * FPC

        def wsect(off):
            w = w_attn[:, off + f0 : off + f0 + FPC]
            return np.ascontiguousarray(
                w.reshape(NCB, P, FPC).transpose(1, 0, 2)
            ).astype(BF)

        in_maps.append(
            {
                "x": xT[b],
                "wq": wsect(0),
                "wk": wsect(C),
                "wv": wsect(2 * C),
                "wp": np.ascontiguousarray(
                    w_proj[f0 : f0 + FPC, :].reshape(2, P, C).transpose(1, 0, 2)
                ).astype(BF),
                "bq": np.ascontiguousarray(
                    b_attn[f0 : f0 + FPC].reshape(2, P).T
                ).astype(np.float32),
                "bk": np.ascontiguousarray(
                    b_attn[C + f0 : C + f0 + FPC].reshape(2, P).T
                ).astype(np.float32),
                "bv": b_attn[2 * C + f0 : 2 * C + f0 + FPC].reshape(1, FPC).astype(BF),
                "triu": triu,
                "ones1": ones1,
            }
        )

    trace = bool(os.environ.get("BASS_TRACE"))
    res = run_bass_kernel_spmd(
        nc,
        in_maps,
        core_ids=list(range(8)),
        trace=trace,
        tmpdir=os.environ.get("KERNEL_TRACE_DIR") or None,
    )
    LAST_RESULT = res

    y = np.empty((B, T, C), np.float32)
    for b in range(B):
        acc = res.results[4 * b]["out"].astype(np.float32).copy()
        for g in range(1, 4):
            acc += res.results[4 * b + g]["out"]
        y[b] = acc + b_proj[None, :]
    return y


# revision 15
# speedup vs baseline: 2.3813x; 1.1202x over previous
"""Causal self-attention (B=2, T=2048, C=1024, NH=16) on 8 trn2 NeuronCores.

Sharding: core c handles batch b = c//4 and head group g = c%4 (4 heads,
256 features). Each core computes q/k/v for its heads, causal attention in
S^T layout (keys on partitions, queries on the free dim), and a partial
output projection  y_heads @ w_proj[head_rows, :].  The host sums the four
partial projections per batch and adds b_proj.

v3 design notes:
  - All matmul operands bf16 (PSUM accumulation fp32); x is transposed and
    all layouts pre-arranged on the host so every DMA is contiguous.
  - V tiles carry a 64-wide block of ones per head, so the PV matmul drops
    the softmax denominator onto PSUM partitions 64:128 (64 copies of it):
    the reciprocal runs as a parallel [64,512] DVE op and feeds the
    normalizing multiply directly -- no single-partition serial ops, no
    partition-broadcast DMA.
  - Per k-tile the two packed heads' scores land in one 2-bank PSUM tile
    and are exponentiated by a single wide scalar-engine instruction
    (scale=1/8 folded in). Causal masks multiply on the Pool engine.
  - Emission interleaves QKV(qc+1)/proj(qc-1) work-units into the attention
    ki-stream, and PV trails exp by PVLAG steps, so the in-order PE queue
    never parks behind a scalar-engine dependency (keeps the HAM clock
    gate released at 2.4 GHz).
"""

import os
import sys
from collections import deque

import numpy as np
import ml_dtypes

for _p in ("/opt/trn_rl_repo", "/root/.axon_site/_ro/trn_rl_repo"):
    if _p not in sys.path and os.path.isdir(_p):
        sys.path.append(_p)

import concourse.bass as bass  # noqa: E402
import concourse.tile as tile  # noqa: E402
from concourse import bacc, mybir  # noqa: E402
from concourse.bass_utils import run_bass_kernel_spmd  # noqa: E402

P = 128
B, T, C = 2, 2048, 1024
NH, HD = 16, 64
HPC = 4  # heads per core
FPC = HPC * HD  # features per core (256)
QCW = 512  # query-chunk width (PSUM bank = 512 fp32)
NQC = T // QCW
NT = T // P
NCB = C // P
PVLAG = 2  # ki-steps PV trails exp by
F32 = mybir.dt.float32
BF16 = mybir.dt.bfloat16
BF = ml_dtypes.bfloat16
EXP = mybir.ActivationFunctionType.Exp


def build_nc():
    nc = bacc.Bacc("TRN2", target_bir_lowering=False, debug=False)

    x_d = nc.dram_tensor("x", [C, T], BF16, kind="ExternalInput")  # x^T
    wq_d = nc.dram_tensor("wq", [P, NCB, FPC], BF16, kind="ExternalInput")
    wk_d = nc.dram_tensor("wk", [P, NCB, FPC], BF16, kind="ExternalInput")
    wv_d = nc.dram_tensor("wv", [P, NCB, FPC], BF16, kind="ExternalInput")
    wp_d = nc.dram_tensor("wp", [P, 2, C], BF16, kind="ExternalInput")
    bq_d = nc.dram_tensor("bq", [P, 2], F32, kind="ExternalInput")
    bk_d = nc.dram_tensor("bk", [P, 2], F32, kind="ExternalInput")
    bv_d = nc.dram_tensor("bv", [1, FPC], BF16, kind="ExternalInput")
    triu_d = nc.dram_tensor("triu", [P, P], BF16, kind="ExternalInput")
    ones1_d = nc.dram_tensor("ones1", [1, P], BF16, kind="ExternalInput")
    out_d = nc.dram_tensor("out", [T, C], BF16, kind="ExternalOutput")

    from contextlib import ExitStack

    with tile.TileContext(nc) as tc, ExitStack() as ctx:
        consts = ctx.enter_context(tc.tile_pool(name="consts", bufs=1))
        bigs = ctx.enter_context(tc.tile_pool(name="bigs", bufs=1))
        xts = ctx.enter_context(tc.tile_pool(name="xts", bufs=NCB))
        exps = ctx.enter_context(tc.tile_pool(name="exps", bufs=5))
        recs = ctx.enter_context(tc.tile_pool(name="recs", bufs=2))
        stage = ctx.enter_context(tc.tile_pool(name="stage", bufs=3))
        psum = ctx.enter_context(tc.tile_pool(name="psum", bufs=2, space="PSUM"))

        # ---- weights / consts into SBUF (all DMAs fully contiguous) ----
        wq_sb = bigs.tile([P, NCB, FPC], BF16, tag="wq")
        wk_sb = bigs.tile([P, NCB, FPC], BF16, tag="wk")
        nc.sync.dma_start(out=wq_sb, in_=wq_d.ap())
        nc.sync.dma_start(out=wk_sb, in_=wk_d.ap())
        xt = [xts.tile([P, T], BF16, tag="xt", name=f"xt{i}") for i in range(NCB)]
        for cb in range(NCB):
            nc.sync.dma_start(out=xt[cb], in_=x_d.ap()[cb * P : (cb + 1) * P, :])
        wv_sb = bigs.tile([P, NCB, FPC], BF16, tag="wv")
        wp_sb = bigs.tile([P, 2, C], BF16, tag="wp")
        nc.sync.dma_start(out=wv_sb, in_=wv_d.ap())
        nc.sync.dma_start(out=wp_sb, in_=wp_d.ap())
        bq_sb = consts.tile([P, 2], F32, tag="bq")
        bk_sb = consts.tile([P, 2], F32, tag="bk")
        bv_sb = consts.tile([1, FPC], BF16, tag="bv")
        triu = consts.tile([P, P], BF16, tag="triu")
        ones1 = consts.tile([1, P], BF16, tag="ones1")
        for t_, d_ in ((bq_sb, bq_d), (bk_sb, bk_d), (bv_sb, bv_d),
                       (triu, triu_d), (ones1, ones1_d)):
            nc.sync.dma_start(out=t_, in_=d_.ap())

        qt = [bigs.tile([P, T], BF16, tag=f"qt{i}", name=f"qt{i}") for i in range(2)]
        kt = [bigs.tile([P, T], BF16, tag=f"kt{i}", name=f"kt{i}") for i in range(2)]
        yt = [bigs.tile([P, T], BF16, tag=f"yt{i}", name=f"yt{i}") for i in range(2)]
        # V layout [P(t-rows), NT, pair, head, 128]: per head cols 0:64 hold
        # v, cols 64:128 hold ones.  PV with this 128-wide stationary tile
        # puts y on PSUM rows 0:64 and 64 copies of the softmax denominator
        # on rows 64:128 (so 1/sum is a parallel 64-partition DVE op).
        v_sb = bigs.tile([P, NT, 2, 2, P], BF16, tag="v")
        nc.gpsimd.memset(
            v_sb.rearrange("p t a h w -> p (t a h) w")[:, :, 64:P], 1.0
        )

        # ---------- work-unit emitters ----------
        def qk_group(qc, wsb, bsb, dst, ft, tag="mm"):
            cs = slice(qc * QCW, (qc + 1) * QCW)
            ps = psum.tile([P, QCW], F32, tag=tag, name="qk", bufs=2)
            for cb in range(NCB):
                nc.tensor.matmul(
                    ps, wsb[:, cb, ft * P : (ft + 1) * P], xt[cb][:, cs],
                    start=(cb == 0), stop=(cb == NCB - 1),
                )
            nc.vector.tensor_scalar_add(dst[ft][:, cs], ps, bsb[:, ft : ft + 1])

        def v_unit(t):
            ps = psum.tile([P, FPC], F32, tag="mm", name="v", bufs=2)
            for cb in range(NCB):
                nc.tensor.matmul(
                    ps, xt[cb][:, t * P : (t + 1) * P], wv_sb[:, cb, :],
                    start=(cb == 0), stop=False,
                )
            nc.tensor.matmul(ps, ones1, bv_sb, start=False, stop=True)
            nc.vector.tensor_copy(
                out=v_sb[:, t, :, :, 0:64].rearrange("p a h w -> p (a h) w"),
                in_=ps.rearrange("p (a h w) -> p (a h) w", a=2, w=64),
            )

        def proj_unit(t, nch):
            ts = slice(t * P, (t + 1) * P)
            ps = psum.tile([P, QCW], F32, tag="mm", name="proj", bufs=2)
            for fb in range(2):
                nc.tensor.matmul(
                    ps, yt[fb][:, ts], wp_sb[:, fb, nch * QCW : (nch + 1) * QCW],
                    start=(fb == 0), stop=(fb == 1),
                )
            ost = stage.tile([P, QCW], BF16, tag="stage")
            nc.vector.tensor_copy(out=ost, in_=ps)
            nc.sync.dma_start(
                out=out_d.ap()[ts, nch * QCW : (nch + 1) * QCW], in_=ost
            )

        def qkv_units(qc):
            for gi, (wsb, bsb, dst) in enumerate(
                ((wq_sb, bq_sb, qt), (wk_sb, bk_sb, kt))
            ):
                for ft in range(2):
                    yield lambda w=wsb, b=bsb, d=dst, f=ft: qk_group(qc, w, b, d, f)
            for t in range(4 * qc, 4 * qc + 4):
                yield lambda t_=t: v_unit(t_)

        def qk_ramp():
            """QK for qc=0 and q of qc=1, cb-outer across six PSUM slots
            (tags mm/st/y all idle during the DMA ramp): every arriving x^T
            tile immediately feeds six matmuls."""
            g6 = [
                (wq_sb, bq_sb, qt, 0, 0, "mm"), (wq_sb, bq_sb, qt, 1, 0, "mm"),
                (wk_sb, bk_sb, kt, 0, 0, "st"), (wk_sb, bk_sb, kt, 1, 0, "st"),
                (wq_sb, bq_sb, qt, 0, 1, "y"), (wq_sb, bq_sb, qt, 1, 1, "y"),
            ]
            pss = [
                psum.tile([P, QCW], F32, tag=tag, name=f"rqk{i}", bufs=2)
                for i, (_, _, _, _, _, tag) in enumerate(g6)
            ]
            for cb in range(NCB):
                for ps, (wsb, _, _, ft, qc, _) in zip(pss, g6):
                    nc.tensor.matmul(
                        ps, wsb[:, cb, ft * P : (ft + 1) * P],
                        xt[cb][:, qc * QCW : (qc + 1) * QCW],
                        start=(cb == 0), stop=(cb == NCB - 1),
                    )
            for ps, (_, bsb, dst, ft, qc, _) in zip(pss, g6):
                nc.vector.tensor_scalar_add(
                    dst[ft][:, qc * QCW : (qc + 1) * QCW], ps, bsb[:, ft : ft + 1]
                )
            qk_group(1, wk_sb, bk_sb, kt, 0)
            qk_group(1, wk_sb, bk_sb, kt, 1)
            for t in range(4):
                v_unit(t)

        def proj_units(qc):
            for t in range(4 * qc, 4 * qc + 4):
                for nch in range(2):
                    yield lambda t_=t, n_=nch: proj_unit(t_, n_)

        # ---------- attention ----------
        def attn(qc, inj):
            cs = slice(qc * QCW, (qc + 1) * QCW)
            cs0 = qc * QCW
            nki = 4 * (qc + 1)
            for pair in range(2):
                yA = psum.tile([P, QCW], F32, tag="y", name="yA", bufs=2)
                yB = psum.tile([P, QCW], F32, tag="y", name="yB", bufs=2)
                pend = deque()

                def emit_s(ki):
                    m = ki - 4 * qc
                    lo = max(m, 0) * P
                    ks = slice(ki * P, (ki + 1) * P)
                    st = psum.tile([P, 2, QCW], F32, tag="st", name="st", bufs=2)
                    nc.tensor.matmul(
                        st[:, 0, lo:], kt[pair][0:64, ks],
                        qt[pair][0:64, cs0 + lo : cs0 + QCW], start=True, stop=True,
                    )
                    nc.tensor.matmul(
                        st[:, 1, lo:], kt[pair][64:P, ks],
                        qt[pair][64:P, cs0 + lo : cs0 + QCW], start=True, stop=True,
                        tile_position=(64, 0),
                    )
                    e = exps.tile([P, 2, QCW], BF16, tag="exp", name="e")
                    nc.scalar.activation(e[:, :, lo:], st[:, :, lo:], EXP, scale=0.125)
                    if m >= 0:  # diagonal 128-block: causal triangle mask
                        ds_ = slice(m * P, (m + 1) * P)
                        nc.gpsimd.tensor_mul(e[:, 0, ds_], e[:, 0, ds_], triu)
                        nc.gpsimd.tensor_mul(e[:, 1, ds_], e[:, 1, ds_], triu)
                    pend.append((ki, lo, e))

                def emit_pv():
                    ki, lo, e = pend.popleft()
                    st_, sp = ki == 0, ki == nki - 1
                    nc.tensor.matmul(
                        yA[:, lo:], v_sb[:, ki, pair, 0], e[:, 0, lo:],
                        start=st_, stop=sp,
                    )
                    nc.tensor.matmul(
                        yB[:, lo:], v_sb[:, ki, pair, 1], e[:, 1, lo:],
                        start=st_, stop=sp,
                    )

                for ki in range(nki):
                    emit_s(ki)
                    if len(pend) > PVLAG:
                        emit_pv()
                    if ki % 2 == 0 and inj:
                        inj.pop(0)[1]()
                while pend:
                    emit_pv()
                    if inj:
                        inj.pop(0)[1]()

                # normalize: parallel [64,512] reciprocal of the denominator
                # rows, multiply straight out of PSUM into the bf16 y^T tile.
                for half, yps in ((0, yA), (1, yB)):
                    ssum = recs.tile([64, QCW], F32, tag="ssum")
                    nc.vector.tensor_copy(out=ssum, in_=yps[64:P, :])
                    rec = recs.tile([64, QCW], F32, tag="rec")
                    nc.vector.reciprocal_approx_fast(out=rec, in_=ssum)
                    nc.vector.tensor_mul(
                        yt[pair][half * 64 : half * 64 + 64, cs], yps[0:64, :], rec
                    )

        # ---------- program ----------
        # Injection queue items are (need_qc, fn): fn must run before
        # attn(need_qc) starts (in-order PE queue: a PV emitted before its
        # V-producer would deadlock).  proj units carry need_qc=None.
        qk_ramp()
        inj = [(1, (lambda t_=t: v_unit(t_))) for t in range(4, 8)]
        inj += [(2, u) for u in qkv_units(2)]
        for qc in range(NQC):
            i = 0
            while i < len(inj):  # force-drain units attn(qc) depends on
                if inj[i][0] is not None and inj[i][0] <= qc:
                    inj.pop(i)[1]()
                else:
                    i += 1
            attn(qc, inj)
            if qc == 0:
                inj += [(3, u) for u in qkv_units(3)]
            inj += [(None, u) for u in proj_units(qc)]
        while inj:
            inj.pop(0)[1]()

    nc.compile()
    return nc


_NC_CACHE: dict = {}
LAST_RESULT = None


def kernel(x, w_attn, b_attn, w_proj, b_proj):
    global LAST_RESULT
    x = np.asarray(x, np.float32)
    w_attn = np.asarray(w_attn, np.float32)
    b_attn = np.asarray(b_attn, np.float32)
    w_proj = np.asarray(w_proj, np.float32)
    b_proj = np.asarray(b_proj, np.float32)

    if "nc" not in _NC_CACHE:
        _NC_CACHE["nc"] = build_nc()
    nc = _NC_CACHE["nc"]

    triu = np.triu(np.ones((P, P), np.float32)).astype(BF)
    ones1 = np.ones((1, P), BF)
    xT = [np.ascontiguousarray(x[b].T).astype(BF) for b in range(B)]

    in_maps = []
    for core in range(8):
        b, g = core // 4, core % 4
        f0 = g * FPC

        def wsect(off):
            w = w_attn[:, off + f0 : off + f0 + FPC]
            return np.ascontiguousarray(
                w.reshape(NCB, P, FPC).transpose(1, 0, 2)
            ).astype(BF)

        in_maps.append(
            {
                "x": xT[b],
                "wq": wsect(0),
                "wk": wsect(C),
                "wv": wsect(2 * C),
                "wp": np.ascontiguousarray(
                    w_proj[f0 : f0 + FPC, :].reshape(2, P, C).transpose(1, 0, 2)
                ).astype(BF),
                "bq": np.ascontiguousarray(
                    b_attn[f0 : f0 + FPC].reshape(2, P).T
                ).astype(np.float32),
                "bk": np.ascontiguousarray(
                    b_attn[C + f0 : C + f0 + FPC].reshape(2, P).T
                ).astype(np.float32),
                "bv": b_attn[2 * C + f0 : 2 * C + f0 + FPC].reshape(1, FPC).astype(BF),
                "triu": triu,
                "ones1": ones1,
            }
        )

    trace = bool(os.environ.get("BASS_TRACE"))
    res = run_bass_kernel_spmd(
        nc,
        in_maps,
        core_ids=list(range(8)),
        trace=trace,
        tmpdir=os.environ.get("KERNEL_TRACE_DIR") or None,
    )
    LAST_RESULT = res

    y = np.empty((B, T, C), np.float32)
    for b in range(B):
        acc = res.results[4 * b]["out"].astype(np.float32)
        for g in range(1, 4):
            acc += res.results[4 * b + g]["out"].astype(np.float32)
        y[b] = acc + b_proj[None, :]
    return y
